# revision 1
# baseline (speedup 1.0000x reference)
"""CrystalConvLayer (GNN message passing) on 8 Trainium2 NeuronCores.

Strategy (node-partitioned, edge-parallel, zero collectives):
  - Core c owns nodes [c*12544, (c+1)*12544). Edges are routed to the core
    owning their dst node; h is replicated to every core for the src gather.
  - Edge MLP runs as feature-major matmul chains on PE.
  - h[src] gather uses gpsimd dma_gather (int16 indices). Since idx16 only
    covers 32k rows, edges are bucketed by src range (4 buckets of 25000
    rows) and each gather call uses a base-offset slice of h.
  - segment_sum(messages, dst) is a matmul against a one-hot selection
    matrix S[e, n] = (dst_local[e] == n), accumulated in PSUM per
    128-node block; exact fp32 accumulation, no cancellation.
  - Node MLP + residual + LayerNorm run per 128-node tile.
  - SPMD: one program for all 8 cores; per-(bucket,block) segment sizes are
    padded to the max across cores so the instruction stream is uniform.

PE matmuls are limited to ONE sync-wait by walrus codegen, so every PSUM
accumulation group is opened by a rank-1 matmul on constant operands (zeros
or the bias row) which absorbs the PSUM WAR wait; data matmuls then carry at
most one wait each.
"""

import sys

if "/opt/trn_rl_repo" not in sys.path:
    sys.path.insert(0, "/opt/trn_rl_repo")

import numpy as np
from contextlib import ExitStack

from concourse import bacc, bass, mybir, tile
from concourse.bass_utils import run_bass_kernel_spmd

F32 = mybir.dt.float32
I16 = mybir.dt.int16


def _legalize_waits(nc):
    """walrus codegen accepts a single sync-wait per instruction; move any
    extra waits onto same-engine nops inserted immediately before it."""
    k = 0
    for f in nc.m.functions:
        for bb in f.blocks:
            out = []
            for ins in bb.instructions:
                si = ins.sync_info
                if si is not None and len(si.on_wait) > 1:
                    waits = list(si.on_wait)
                    for w in waits[:-1]:
                        k += 1
                        n = mybir.InstNoOp(name=nc.get_next_instruction_name(),
                                           ins=[], outs=[])
                        n.engine = ins.engine
                        n.sync_info = mybir.SyncInfo(on_wait=[w], on_update=[])
                        out.append(n)
                    ins.sync_info = mybir.SyncInfo(
                        on_wait=[waits[-1]], on_update=list(si.on_update))
                out.append(ins)
            bb.instructions = out
    return nc

P = 128
H = 128
TRUNC = 50
EPS = 1e-5
NCORES = 8
GRP = 4          # edge tiles per mm1/psum group (free dim 512)
CH_TILES = 8     # edge tiles per dma_gather call (multiple of GRP);
                 # >=1536 idxs per call crashes the gather ucode on HW

# fraction of edge groups whose mult+S-build run on gpsimd (Pool) instead of
# DVE: group g goes to Pool when (g % POOL_MOD) < POOL_NUM. The mult and all
# four S-builds of a group share one engine so the scatter matmul's waits
# coalesce into a single semaphore.
POOL_NUM, POOL_MOD = 0, 8   # gpsimd is gather-only (mlp ucode library)
CPACK_W = 2432


def _full_cfg(N, E):
    npc = -(-N // (NCORES * P)) * P          # padded nodes per core
    return dict(
        N=N, E=E,
        NPC=npc,
        NBLK=npc // P,
        BUCKET=25000 if N > 25000 else -(-N // 4),
    )


def _prep(cfg, h, edge_index, edge_attr, edge_sh):
    """Host-side sharding: route/sort/pad edges, build gather indices."""
    N, NPC, NBLK, BUCKET = cfg["N"], cfg["NPC"], cfg["NBLK"], cfg["BUCKET"]
    NBUCK = -(-N // BUCKET)
    src = np.asarray(edge_index[0]).astype(np.int64)
    dst = np.asarray(edge_index[1]).astype(np.int64)
    ein = np.ascontiguousarray(
        np.concatenate([edge_attr, edge_sh], axis=1)[:, :TRUNC].astype(np.float32)
    )
    core_of = dst // NPC

    per = []
    cnt = np.zeros((NCORES, NBUCK, NBLK), np.int64)
    for c in range(NCORES):
        sel = np.nonzero(core_of == c)[0]
        dl = dst[sel] - c * NPC
        bk = src[sel] // BUCKET
        bl = dl // P
        o = np.lexsort((bl, bk))
        sel, dl, bk, bl = sel[o], dl[o], bk[o], bl[o]
        np.add.at(cnt[c], (bk, bl), 1)
        per.append((sel, dl))

    T = -(-cnt.max(axis=0) // P)             # [NBUCK, NBLK] tiles per segment
    for b in range(NBUCK):
        r = int(T[b].sum()) % GRP
        if r:
            T[b, NBLK - 1] += GRP - r

    segs = []                                # (bucket, blk, tile_start, ntiles)
    bucket_tiles = []
    tg = 0
    for b in range(NBUCK):
        bt0 = tg
        for k in range(NBLK):
            t = int(T[b, k])
            if t:
                segs.append((b, k, tg, t))
                tg += t
        bucket_tiles.append((bt0, tg - bt0))
    NT = tg
    EP = NT * P

    chunks = []                              # (bucket, tile_start, ntiles)
    for b, (bt0, bn) in enumerate(bucket_tiles):
        t = 0
        while t < bn:
            n = min(CH_TILES, bn - t)
            chunks.append((b, bt0 + t, n))
            t += n

    # tile -> (segment index, is_first, is_last)
    tseg = [None] * NT
    for si, (b, k, ts, t) in enumerate(segs):
        for i in range(t):
            tseg[ts + i] = (si, i == 0, i == t - 1)

    hpad = np.zeros((NCORES * NPC, H), np.float32)
    hpad[:N] = np.asarray(h, np.float32)

    per_core = []
    for c in range(NCORES):
        sel, dl = per[c]
        ein_pad = np.zeros((EP, TRUNC), np.float32)
        gi = np.zeros(EP, np.int16)
        df = np.full(EP, 300.0, np.float32)
        pos = 0
        for (b, k, ts, t) in segs:
            n = int(cnt[c, b, k])
            off = ts * P
            s = sel[pos:pos + n]
            ein_pad[off:off + n] = ein[s]
            gi[off:off + n] = (src[s] - b * BUCKET).astype(np.int16)
            df[off:off + n] = (dl[pos:pos + n] % P).astype(np.float32)
            pos += n
        assert pos == len(sel)
        giw = np.tile(gi.reshape(EP // 16, 16).T, (8, 1))     # [128, EP/16]
        dfw = df.reshape(NT, P).T.copy()                      # [128, NT]
        hs = hpad[c * NPC:(c + 1) * NPC]
        per_core.append(dict(
            einT=np.ascontiguousarray(ein_pad.T),
            gidx=np.ascontiguousarray(giw),
            dstf=np.ascontiguousarray(dfw),
            hTp=np.ascontiguousarray(hs.T),
            hp=np.ascontiguousarray(hs),
        ))

    meta = dict(NBUCK=NBUCK, NT=NT, EP=EP, segs=segs, chunks=chunks, tseg=tseg)
    return meta, per_core, hpad


def _build(cfg, meta, weights, loop=1):
    """Emit the SPMD Bass program.

    Wait discipline: walrus allows very few (treat as one) cross-engine
    sync-waits per compute instruction. Tactics used here:
      - all constants ship in one packed tensor -> one DMA semaphore,
        absorbed per-engine by warm-up touch ops;
      - every PSUM accumulation group opens with a rank-1 matmul on
        constants which absorbs the PSUM WAR release;
      - each gather chunk gets per-engine touch ops so the consumer ops
        only wait on their upstream compute engine;
      - the S one-hot build and the message multiply of a group run on the
        same engine so the scatter matmul sees a single semaphore.
    """
    N, NPC, NBLK, BUCKET = cfg["N"], cfg["NPC"], cfg["NBLK"], cfg["BUCKET"]
    NBUCK, NT, EP = meta["NBUCK"], meta["NT"], meta["EP"]
    segs, chunks, tseg = meta["segs"], meta["chunks"], meta["tseg"]

    nc = bacc.Bacc("TRN2", target_bir_lowering=False, debug=False,
                   num_devices=NCORES)

    h_d = nc.dram_tensor("h", [N, H], F32, kind="ExternalInput")
    einT_d = nc.dram_tensor("einT", [TRUNC, EP], F32, kind="ExternalInput")
    gidx_d = nc.dram_tensor("gidx", [P, EP // 16], I16, kind="ExternalInput")
    dstf_d = nc.dram_tensor("dstf", [P, NT], F32, kind="ExternalInput")
    hTp_d = nc.dram_tensor("hTp", [P, NPC], F32, kind="ExternalInput")
    hp_d = nc.dram_tensor("hp", [NPC, H], F32, kind="ExternalInput")
    cst_d = {k: nc.dram_tensor(k, list(v.shape), F32, kind="ExternalInput")
             for k, v in weights.items()}
    out_d = nc.dram_tensor("out", [NPC, H], F32, kind="ExternalOutput")

    with tile.TileContext(nc) as tc, ExitStack() as ctx:
        cp = ctx.enter_context(tc.tile_pool(name="cst", bufs=1))
        bigp = ctx.enter_context(tc.tile_pool(name="big", bufs=1))
        gp = ctx.enter_context(tc.tile_pool(name="gch", bufs=3))
        ep = ctx.enter_context(tc.tile_pool(name="edge", bufs=3))
        sp_ = ctx.enter_context(tc.tile_pool(name="spool", bufs=6))
        np_ = ctx.enter_context(tc.tile_pool(name="node", bufs=3))
        ps1 = ctx.enter_context(tc.tile_pool(name="ps1", bufs=2, space="PSUM"))
        psw = ctx.enter_context(tc.tile_pool(name="psw", bufs=2, space="PSUM"))
        psa = ctx.enter_context(tc.tile_pool(name="psa", bufs=2, space="PSUM"))
        psn = ctx.enter_context(tc.tile_pool(name="psn", bufs=2, space="PSUM"))

        # ---- constants: one DMA, sliced views ----
        cpk = cp.tile([P, CPACK_W], F32, tag="cpack")
        nc.sync.dma_start(out=cpk[:], in_=cst_d["cpack"][:])
        cst = dict(
            iota=cpk[:, 0:128], lng=cpk[:, 128:256], lnb=cpk[:, 256:384],
            b1e=cpk[:, 384:385], b1n=cpk[:, 385:386], epsc=cpk[:, 386:387],
            zrow=cpk[0:1, 512:1024], b2e4=cpk[0:1, 1024:1536],
            b2nr=cpk[0:1, 1536:1664], ocol=cpk[0:1, 1664:1792],
            zcol=cpk[0:1, 512:640],
            W1e=cpk[0:TRUNC, 1792:1920], W2e=cpk[:, 1920:2048],
            W1na=cpk[:, 2048:2176], W1nb=cpk[:, 2176:2304],
            W2n=cpk[:, 2304:2432],
        )
        gidx_sb = bigp.tile([P, EP // 16], I16)
        nc.sync.dma_start(out=gidx_sb[:], in_=gidx_d[:])
        dstf_sb = bigp.tile([P, NT], F32)
        nc.sync.dma_start(out=dstf_sb[:], in_=dstf_d[:])
        aggT = bigp.tile([P, NPC], F32)

        def opener(out_ap, rhs_ap=None, lhs_ap=None):
            nc.tensor.matmul(
                out=out_ap,
                lhsT=(lhs_ap if lhs_ap is not None else cst["zcol"]),
                rhs=(rhs_ap if rhs_ap is not None else
                     cst["zrow"][:, :out_ap.shape[-1]]),
                start=True, stop=False, skip_group_check=True)

        def emit_body():
            _emit(nc, tc, cfg, meta, cst, cpk, gidx_sb, dstf_sb, aggT, opener,
                  h_d, einT_d, hTp_d, hp_d, out_d,
                  gp, ep, sp_, np_, ps1, psw, psa, psn)

        if loop > 1:
            with tc.For_i(0, loop, 1):
                emit_body()
        else:
            emit_body()

    return nc


def _emit(nc, tc, cfg, meta, cst, cpk, gidx_sb, dstf_sb, aggT, opener,
          h_d, einT_d, hTp_d, hp_d, out_d,
          gp, ep, sp_, np_, ps1, psw, psa, psn):
        N, NPC, NBLK, BUCKET = cfg["N"], cfg["NPC"], cfg["NBLK"], cfg["BUCKET"]
        NBUCK, NT, EP = meta["NBUCK"], meta["NT"], meta["EP"]
        segs, chunks, tseg = meta["segs"], meta["chunks"], meta["tseg"]
        nc.vector.memset(aggT[:], 0.0)
        # ---- phase E: edge MLP + gather + scatter-matmul ----
        seg_psum = {}
        gch = None
        gch_t0 = 0
        ci = 0
        for g in range(NT // GRP):
            t0 = g * GRP
            pool_grp = (g % POOL_MOD) < POOL_NUM
            if ci < len(chunks) and chunks[ci][1] == t0:
                b, ts, nt = chunks[ci]
                ci += 1
                gch = gp.tile([P, CH_TILES, H], F32, tag="gch")
                gch_t0 = ts
                base = b * BUCKET
                rows = min(BUCKET, N - base)
                nc.gpsimd.dma_gather(
                    out_ap=gch[:, :nt, :],
                    in_ap=h_d[base:base + rows, :],
                    idxs_ap=gidx_sb[:, ts * 8:(ts + nt) * 8],
                    num_idxs=nt * P,
                    num_idxs_reg=nt * P,
                    elem_size=H,
                )
            ein_t = ep.tile([TRUNC, GRP * P], F32, tag="ein")
            nc.sync.dma_start(out=ein_t[:], in_=einT_d[:, t0 * P:(t0 + GRP) * P])

            p1 = ps1.tile([P, GRP * P], F32, space="PSUM", tag="p1")
            nc.tensor.matmul(out=p1[:], lhsT=cst["W1e"], rhs=ein_t[:],
                             start=True, stop=True, skip_group_check=True)
            hidT = ep.tile([P, GRP * P], F32, tag="hidT")
            nc.scalar.activation(out=hidT[:], in_=p1[:],
                                 func=mybir.ActivationFunctionType.Silu,
                                 bias=cst["b1e"])

            pw = psw.tile([P, GRP * P], F32, space="PSUM", tag="pw")
            opener(pw[:], rhs_ap=cst["b2e4"], lhs_ap=cst["ocol"])
            for i in range(GRP):
                nc.tensor.matmul(out=pw[:, i * P:(i + 1) * P],
                                 lhsT=hidT[:, i * P:(i + 1) * P],
                                 rhs=cst["W2e"],
                                 start=False, stop=(i == GRP - 1),
                                 skip_group_check=True)

            msg = ep.tile([P, GRP * P], F32, tag="msg")
            o = t0 - gch_t0
            gin = gch[:, o:o + GRP, :].rearrange("p a b -> p (a b)")
            if pool_grp:
                # gpsimd cannot read PSUM: bounce w_edge through SBUF (ACT).
                wsb = ep.tile([P, GRP * P], F32, tag="wsb")
                nc.scalar.copy(out=wsb[:], in_=pw[:])
                nc.gpsimd.tensor_tensor(out=msg[:], in0=wsb[:], in1=gin,
                                        op=mybir.AluOpType.mult)
                seng = nc.gpsimd
            else:
                nc.vector.tensor_tensor(out=msg[:], in0=pw[:], in1=gin,
                                        op=mybir.AluOpType.mult)
                seng = nc.vector

            for i in range(GRP):
                t = t0 + i
                si, first, last = tseg[t]
                blk = segs[si][1]
                s_t = sp_.tile([P, P], F32, tag="S")
                seng.tensor_scalar(out=s_t[:], in0=cst["iota"],
                                   scalar1=dstf_sb[:, t:t + 1], scalar2=None,
                                   op0=mybir.AluOpType.is_equal)
                if first:
                    pa = psa.tile([P, P], F32, space="PSUM", tag="pagg")
                    seg_psum[si] = pa
                pa = seg_psum[si]
                nc.tensor.matmul(out=pa[:], lhsT=msg[:, i * P:(i + 1) * P],
                                 rhs=s_t[:], start=first, stop=last,
                                 skip_group_check=True)
                if last:
                    nc.vector.tensor_tensor(
                        out=aggT[:, blk * P:(blk + 1) * P],
                        in0=aggT[:, blk * P:(blk + 1) * P],
                        in1=pa[:], op=mybir.AluOpType.add)
                    del seg_psum[si]

        # ---- phase N: node MLP + residual + LayerNorm ----
        for j in range(NBLK):
            hT_t = np_.tile([P, P], F32, tag="hT")
            nc.sync.dma_start(out=hT_t[:], in_=hTp_d[:, j * P:(j + 1) * P])
            h_t = np_.tile([P, P], F32, tag="hn")
            nc.sync.dma_start(out=h_t[:], in_=hp_d[j * P:(j + 1) * P, :])

            pn = psn.tile([P, P], F32, space="PSUM", tag="pno")
            nc.tensor.matmul(out=pn[:], lhsT=cst["W1na"], rhs=hT_t[:],
                             start=True, stop=False, skip_group_check=True)
            nc.tensor.matmul(out=pn[:], lhsT=cst["W1nb"],
                             rhs=aggT[:, j * P:(j + 1) * P],
                             start=False, stop=True, skip_group_check=True)
            hidn = np_.tile([P, P], F32, tag="hidn")
            nc.scalar.activation(out=hidn[:], in_=pn[:],
                                 func=mybir.ActivationFunctionType.Silu,
                                 bias=cst["b1n"])

            po = psn.tile([P, P], F32, space="PSUM", tag="pno")
            opener(po[:], rhs_ap=cst["b2nr"], lhs_ap=cst["ocol"])
            nc.tensor.matmul(out=po[:], lhsT=hidn[:], rhs=cst["W2n"],
                             start=False, stop=True, skip_group_check=True)

            x = np_.tile([P, P], F32, tag="x")
            nc.vector.tensor_tensor(out=x[:], in0=po[:], in1=h_t[:],
                                    op=mybir.AluOpType.add)
            st = np_.tile([P, 4], F32, tag="st")
            nc.vector.tensor_reduce(out=st[:, 0:1], in_=x[:],
                                    axis=mybir.AxisListType.X,
                                    op=mybir.AluOpType.add)
            nc.scalar.activation(out=st[:, 1:2], in_=st[:, 0:1],
                                 func=mybir.ActivationFunctionType.Copy,
                                 scale=1.0 / H)
            xm = np_.tile([P, P], F32, tag="xm")
            nc.vector.tensor_scalar(out=xm[:], in0=x[:],
                                    scalar1=st[:, 1:2], scalar2=None,
                                    op0=mybir.AluOpType.subtract)
            sq = np_.tile([P, P], F32, tag="sq")
            nc.scalar.activation(out=sq[:], in_=xm[:],
                                 func=mybir.ActivationFunctionType.Square,
                                 accum_out=st[:, 2:3])
            nc.scalar.activation(out=st[:, 3:4], in_=st[:, 2:3],
                                 func=mybir.ActivationFunctionType.Sqrt,
                                 scale=1.0 / H, bias=cst["epsc"])
            rs = np_.tile([P, 1], F32, tag="rs")
            nc.vector.reciprocal(out=rs[:], in_=st[:, 3:4])
            y = np_.tile([P, P], F32, tag="y")
            nc.vector.scalar_tensor_tensor(
                out=y[:], in0=xm[:], scalar=rs[:, 0:1], in1=cst["lng"],
                op0=mybir.AluOpType.mult, op1=mybir.AluOpType.mult)
            yo = np_.tile([P, P], F32, tag="yo")
            nc.vector.tensor_tensor(out=yo[:], in0=y[:], in1=cst["lnb"],
                                    op=mybir.AluOpType.add)
            nc.sync.dma_start(out=out_d[j * P:(j + 1) * P, :], in_=yo[:])


def _prepare(cfg, h, edge_index, edge_attr, edge_sh,
             W1e, b1e, W2e, b2e, W1n, b1n, W2n, b2n, ln_g, ln_b,
             loop=1):
    meta, per_core, _ = _prep(cfg, h, edge_index, edge_attr, edge_sh)
    N, NPC = cfg["N"], cfg["NPC"]

    cpack = np.zeros((P, CPACK_W), np.float32)
    cpack[:, 0:128] = np.tile(np.arange(P, dtype=np.float32)[None, :], (P, 1))
    cpack[:, 128:256] = np.tile(np.asarray(ln_g, np.float32).reshape(1, -1), (P, 1))
    cpack[:, 256:384] = np.tile(np.asarray(ln_b, np.float32).reshape(1, -1), (P, 1))
    cpack[:, 384] = np.asarray(b1e, np.float32)
    cpack[:, 385] = np.asarray(b1n, np.float32)
    cpack[:, 386] = EPS
    # row-0 vectors (zrow at 512:1024 stays zero)
    cpack[0, 1024:1536] = np.tile(np.asarray(b2e, np.float32), GRP)
    cpack[0, 1536:1664] = np.asarray(b2n, np.float32)
    cpack[0, 1664:1792] = 1.0
    W1n32 = np.asarray(W1n, np.float32)
    cpack[0:TRUNC, 1792:1920] = np.asarray(W1e, np.float32)
    cpack[:, 1920:2048] = np.asarray(W2e, np.float32)
    cpack[:, 2048:2176] = W1n32[:H]
    cpack[:, 2176:2304] = W1n32[H:]
    cpack[:, 2304:2432] = np.asarray(W2n, np.float32)
    weights = dict(cpack=cpack)

    nc = _build(cfg, meta, weights, loop=loop)
    nc.compile()

    h32 = np.ascontiguousarray(np.asarray(h, np.float32))
    in_maps = []
    for c in range(NCORES):
        m = dict(h=h32, **per_core[c], **weights)
        in_maps.append(m)
    return nc, in_maps


def _run(cfg, h, edge_index, edge_attr, edge_sh,
         W1e, b1e, W2e, b2e, W1n, b1n, W2n, b2n, ln_g, ln_b, trace=False):
    nc, in_maps = _prepare(cfg, h, edge_index, edge_attr, edge_sh,
                           W1e, b1e, W2e, b2e, W1n, b1n, W2n, b2n, ln_g, ln_b)
    res = run_bass_kernel_spmd(nc, in_maps, list(range(NCORES)), trace=trace)
    out = np.concatenate([res.results[c]["out"] for c in range(NCORES)], axis=0)
    return out[:cfg["N"]], res


def kernel(h, edge_index, edge_attr, edge_sh,
           W1e, b1e, W2e, b2e, W1n, b1n, W2n, b2n, ln_g, ln_b):
    cfg = _full_cfg(N=h.shape[0], E=edge_index.shape[1])
    out, _ = _run(cfg, h, edge_index, edge_attr, edge_sh,
                  W1e, b1e, W2e, b2e, W1n, b1n, W2n, b2n, ln_g, ln_b)
    return out



# revision 3
# speedup vs baseline: 1.3854x; 1.3854x over previous
"""CrystalConvLayer (GNN message passing) on 8 Trainium2 NeuronCores.

Strategy (node-partitioned, edge-parallel, zero collectives):
  - Core c owns nodes [c*12544, (c+1)*12544). Edges are routed to the core
    owning their dst node; h is replicated to every core for the src gather.
  - Edge MLP runs as feature-major matmul chains on PE in bf16 (1 cycle/row
    vs 4 for fp32); PSUM accumulation stays fp32.
  - h[src] gather uses gpsimd dma_gather (int16 indices) on a bf16 copy of
    h (256B rows, half the HBM traffic of f32). Since idx16 only covers 32k
    rows, edges are bucketed by src range (4 buckets of 25000 rows).
  - segment_sum(messages, dst) is a matmul against a bf16 one-hot selection
    matrix S[e, n] = (dst_local[e] == n), accumulated in fp32 PSUM per
    128-node block.
  - Node MLP (bf16 matmuls) + residual + LayerNorm (fp32) per 128-node tile.
  - SPMD: one program for all 8 cores; per-(bucket,block) segment sizes are
    padded to the max across cores so the instruction stream is uniform.

PE matmuls are limited to ONE sync-wait by walrus codegen, so every PSUM
accumulation group is opened by a rank-1 matmul on constant operands (zeros
or the bias row) which absorbs the PSUM WAR wait; data matmuls then carry at
most one wait each.
"""

import sys

if "/opt/trn_rl_repo" not in sys.path:
    sys.path.insert(0, "/opt/trn_rl_repo")

import numpy as np
import ml_dtypes
from contextlib import ExitStack

from concourse import bacc, bass, mybir, tile
from concourse.bass_utils import run_bass_kernel_spmd

F32 = mybir.dt.float32
BF16 = mybir.dt.bfloat16
I16 = mybir.dt.int16
NPBF = ml_dtypes.bfloat16

P = 128
H = 128
TRUNC = 50
EPS = 1e-5
NCORES = 8
GRP = 4          # edge tiles per mm1/psum group (free dim 512)
CH_TILES = 8     # edge tiles per dma_gather call (multiple of GRP);
                 # >=1536 idxs per call crashes the gather ucode on HW

POOL_NUM, POOL_MOD = 0, 8   # gpsimd is gather-only (mlp ucode library)
CPACK_W = 512    # f32 constant pack width
CPBF_W = 2048    # bf16 constant pack width


def _full_cfg(N, E):
    npc = -(-N // (NCORES * P)) * P          # padded nodes per core
    return dict(
        N=N, E=E,
        NPC=npc,
        NBLK=npc // P,
        BUCKET=25000 if N > 25000 else -(-N // 4),
    )


def _bf(x):
    return np.ascontiguousarray(np.asarray(x, np.float32).astype(NPBF))


def _prep(cfg, h, edge_index, edge_attr, edge_sh):
    """Host-side sharding: route/sort/pad edges, build gather indices."""
    N, NPC, NBLK, BUCKET = cfg["N"], cfg["NPC"], cfg["NBLK"], cfg["BUCKET"]
    NBUCK = -(-N // BUCKET)
    src = np.asarray(edge_index[0]).astype(np.int64)
    dst = np.asarray(edge_index[1]).astype(np.int64)
    ein = np.ascontiguousarray(
        np.concatenate([edge_attr, edge_sh], axis=1)[:, :TRUNC].astype(np.float32)
    )
    core_of = dst // NPC

    per = []
    cnt = np.zeros((NCORES, NBUCK, NBLK), np.int64)
    for c in range(NCORES):
        sel = np.nonzero(core_of == c)[0]
        dl = dst[sel] - c * NPC
        bk = src[sel] // BUCKET
        bl = dl // P
        o = np.lexsort((bl, bk))
        sel, dl, bk, bl = sel[o], dl[o], bk[o], bl[o]
        np.add.at(cnt[c], (bk, bl), 1)
        per.append((sel, dl))

    T = -(-cnt.max(axis=0) // P)             # [NBUCK, NBLK] tiles per segment
    for b in range(NBUCK):
        r = int(T[b].sum()) % GRP
        if r:
            T[b, NBLK - 1] += GRP - r

    segs = []                                # (bucket, blk, tile_start, ntiles)
    bucket_tiles = []
    tg = 0
    for b in range(NBUCK):
        bt0 = tg
        for k in range(NBLK):
            t = int(T[b, k])
            if t:
                segs.append((b, k, tg, t))
                tg += t
        bucket_tiles.append((bt0, tg - bt0))
    NT = tg
    EP = NT * P

    chunks = []                              # (bucket, tile_start, ntiles)
    for b, (bt0, bn) in enumerate(bucket_tiles):
        t = 0
        while t < bn:
            n = min(CH_TILES, bn - t)
            chunks.append((b, bt0 + t, n))
            t += n

    # tile -> (segment index, is_first, is_last)
    tseg = [None] * NT
    for si, (b, k, ts, t) in enumerate(segs):
        for i in range(t):
            tseg[ts + i] = (si, i == 0, i == t - 1)

    hpad = np.zeros((NCORES * NPC, H), np.float32)
    hpad[:N] = np.asarray(h, np.float32)

    per_core = []
    for c in range(NCORES):
        sel, dl = per[c]
        ein_pad = np.zeros((EP, TRUNC), np.float32)
        gi = np.zeros(EP, np.int16)
        df = np.full(EP, 300.0, np.float32)
        pos = 0
        for (b, k, ts, t) in segs:
            n = int(cnt[c, b, k])
            off = ts * P
            s = sel[pos:pos + n]
            ein_pad[off:off + n] = ein[s]
            gi[off:off + n] = (src[s] - b * BUCKET).astype(np.int16)
            df[off:off + n] = (dl[pos:pos + n] % P).astype(np.float32)
            pos += n
        assert pos == len(sel)
        giw = np.tile(gi.reshape(EP // 16, 16).T, (8, 1))     # [128, EP/16]
        dfw = df.reshape(NT, P).T.copy()                      # [128, NT]
        hs = hpad[c * NPC:(c + 1) * NPC]
        per_core.append(dict(
            einT=_bf(ein_pad.T),
            gidx=np.ascontiguousarray(giw),
            dstf=np.ascontiguousarray(dfw),
            hTp=_bf(hs.T),
            hp=np.ascontiguousarray(hs),
        ))

    hbf = _bf(np.asarray(h, np.float32))
    meta = dict(NBUCK=NBUCK, NT=NT, EP=EP, segs=segs, chunks=chunks, tseg=tseg)
    return meta, per_core, hbf


def _build(cfg, meta, weights, loop=1):
    """Emit the SPMD Bass program (see module docstring for wait discipline)."""
    N, NPC, NBLK, BUCKET = cfg["N"], cfg["NPC"], cfg["NBLK"], cfg["BUCKET"]
    NBUCK, NT, EP = meta["NBUCK"], meta["NT"], meta["EP"]
    segs, chunks, tseg = meta["segs"], meta["chunks"], meta["tseg"]

    nc = bacc.Bacc("TRN2", target_bir_lowering=False, debug=False,
                   num_devices=NCORES)

    hbf_d = nc.dram_tensor("hbf", [N, H], BF16, kind="ExternalInput")
    einT_d = nc.dram_tensor("einT", [TRUNC, EP], BF16, kind="ExternalInput")
    gidx_d = nc.dram_tensor("gidx", [P, EP // 16], I16, kind="ExternalInput")
    dstf_d = nc.dram_tensor("dstf", [P, NT], F32, kind="ExternalInput")
    hTp_d = nc.dram_tensor("hTp", [P, NPC], BF16, kind="ExternalInput")
    hp_d = nc.dram_tensor("hp", [NPC, H], F32, kind="ExternalInput")
    cst_d = {k: nc.dram_tensor(k, list(v.shape),
                               BF16 if v.dtype == NPBF else F32,
                               kind="ExternalInput")
             for k, v in weights.items()}
    out_d = nc.dram_tensor("out", [NPC, H], F32, kind="ExternalOutput")

    with tile.TileContext(nc) as tc, ExitStack() as ctx:
        cp = ctx.enter_context(tc.tile_pool(name="cst", bufs=1))
        bigp = ctx.enter_context(tc.tile_pool(name="big", bufs=1))
        gp = ctx.enter_context(tc.tile_pool(name="gch", bufs=3))
        ep = ctx.enter_context(tc.tile_pool(name="edge", bufs=3))
        sp_ = ctx.enter_context(tc.tile_pool(name="spool", bufs=6))
        np_ = ctx.enter_context(tc.tile_pool(name="node", bufs=3))
        ps1 = ctx.enter_context(tc.tile_pool(name="ps1", bufs=2, space="PSUM"))
        psw = ctx.enter_context(tc.tile_pool(name="psw", bufs=2, space="PSUM"))
        psa = ctx.enter_context(tc.tile_pool(name="psa", bufs=2, space="PSUM"))
        psn = ctx.enter_context(tc.tile_pool(name="psn", bufs=2, space="PSUM"))

        # ---- constants: two DMAs (f32 + bf16 packs), sliced views ----
        cpk = cp.tile([P, CPACK_W], F32, tag="cpack")
        nc.sync.dma_start(out=cpk[:], in_=cst_d["cpack"][:])
        cpb = cp.tile([P, CPBF_W], BF16, tag="cpbf")
        nc.sync.dma_start(out=cpb[:], in_=cst_d["cpbf"][:])
        cst = dict(
            lng=cpk[:, 0:128], lnb=cpk[:, 128:256],
            b1e=cpk[:, 256:257], b1n=cpk[:, 257:258], epsc=cpk[:, 258:259],
            iota=cpb[:, 0:128], W2e=cpb[:, 128:256],
            W1na=cpb[:, 256:384], W1nb=cpb[:, 384:512], W2n=cpb[:, 512:640],
            W1e=cpb[0:TRUNC, 640:768],
            zrow=cpb[0:1, 768:1280], zcol=cpb[0:1, 768:896],
            b2e4=cpb[0:1, 1280:1792], b2nr=cpb[0:1, 1792:1920],
            ocol=cpb[0:1, 1920:2048],
        )
        gidx_sb = bigp.tile([P, EP // 16], I16)
        nc.sync.dma_start(out=gidx_sb[:], in_=gidx_d[:])
        dstf_sb = bigp.tile([P, NT], F32)
        nc.sync.dma_start(out=dstf_sb[:], in_=dstf_d[:])
        aggT = bigp.tile([P, NPC], F32)

        def opener(out_ap, rhs_ap=None, lhs_ap=None):
            nc.tensor.matmul(
                out=out_ap,
                lhsT=(lhs_ap if lhs_ap is not None else cst["zcol"]),
                rhs=(rhs_ap if rhs_ap is not None else
                     cst["zrow"][:, :out_ap.shape[-1]]),
                start=True, stop=False, skip_group_check=True)

        def emit_body():
            _emit(nc, tc, cfg, meta, cst, gidx_sb, dstf_sb, aggT, opener,
                  hbf_d, einT_d, hTp_d, hp_d, out_d,
                  gp, ep, sp_, np_, ps1, psw, psa, psn)

        if loop > 1:
            with tc.For_i(0, loop, 1):
                emit_body()
        else:
            emit_body()

    return nc


def _emit(nc, tc, cfg, meta, cst, gidx_sb, dstf_sb, aggT, opener,
          hbf_d, einT_d, hTp_d, hp_d, out_d,
          gp, ep, sp_, np_, ps1, psw, psa, psn):
        N, NPC, NBLK, BUCKET = cfg["N"], cfg["NPC"], cfg["NBLK"], cfg["BUCKET"]
        NBUCK, NT, EP = meta["NBUCK"], meta["NT"], meta["EP"]
        segs, chunks, tseg = meta["segs"], meta["chunks"], meta["tseg"]
        nc.vector.memset(aggT[:], 0.0)
        # ---- phase E: edge MLP + gather + scatter-matmul ----
        seg_psum = {}
        gch = None
        gch_t0 = 0
        ci = 0
        for g in range(NT // GRP):
            t0 = g * GRP
            pool_grp = (g % POOL_MOD) < POOL_NUM
            if ci < len(chunks) and chunks[ci][1] == t0:
                b, ts, nt = chunks[ci]
                ci += 1
                gch = gp.tile([P, CH_TILES, H], BF16, tag="gch")
                gch_t0 = ts
                base = b * BUCKET
                rows = min(BUCKET, N - base)
                nc.gpsimd.dma_gather(
                    out_ap=gch[:, :nt, :],
                    in_ap=hbf_d[base:base + rows, :],
                    idxs_ap=gidx_sb[:, ts * 8:(ts + nt) * 8],
                    num_idxs=nt * P,
                    num_idxs_reg=nt * P,
                    elem_size=H,
                )
            ein_t = ep.tile([TRUNC, GRP * P], BF16, tag="ein")
            nc.sync.dma_start(out=ein_t[:], in_=einT_d[:, t0 * P:(t0 + GRP) * P])

            p1 = ps1.tile([P, GRP * P], F32, space="PSUM", tag="p1")
            nc.tensor.matmul(out=p1[:], lhsT=cst["W1e"], rhs=ein_t[:],
                             start=True, stop=True, skip_group_check=True)
            hidT = ep.tile([P, GRP * P], BF16, tag="hidT")
            nc.scalar.activation(out=hidT[:], in_=p1[:],
                                 func=mybir.ActivationFunctionType.Silu,
                                 bias=cst["b1e"])

            pw = psw.tile([P, GRP * P], F32, space="PSUM", tag="pw")
            opener(pw[:], rhs_ap=cst["b2e4"], lhs_ap=cst["ocol"])
            for i in range(GRP):
                nc.tensor.matmul(out=pw[:, i * P:(i + 1) * P],
                                 lhsT=hidT[:, i * P:(i + 1) * P],
                                 rhs=cst["W2e"],
                                 start=False, stop=(i == GRP - 1),
                                 skip_group_check=True)

            msg = ep.tile([P, GRP * P], BF16, tag="msg")
            o = t0 - gch_t0
            gin = gch[:, o:o + GRP, :].rearrange("p a b -> p (a b)")
            if pool_grp:
                # gpsimd cannot read PSUM: bounce w_edge through SBUF (ACT).
                wsb = ep.tile([P, GRP * P], BF16, tag="wsb")
                nc.scalar.copy(out=wsb[:], in_=pw[:])
                nc.gpsimd.tensor_tensor(out=msg[:], in0=wsb[:], in1=gin,
                                        op=mybir.AluOpType.mult)
                seng = nc.gpsimd
            else:
                nc.vector.tensor_tensor(out=msg[:], in0=pw[:], in1=gin,
                                        op=mybir.AluOpType.mult)
                seng = nc.vector

            for i in range(GRP):
                t = t0 + i
                si, first, last = tseg[t]
                blk = segs[si][1]
                s_t = sp_.tile([P, P], BF16, tag="S")
                seng.tensor_scalar(out=s_t[:], in0=cst["iota"],
                                   scalar1=dstf_sb[:, t:t + 1], scalar2=None,
                                   op0=mybir.AluOpType.is_equal)
                if first:
                    pa = psa.tile([P, P], F32, space="PSUM", tag="pagg")
                    seg_psum[si] = pa
                pa = seg_psum[si]
                nc.tensor.matmul(out=pa[:], lhsT=msg[:, i * P:(i + 1) * P],
                                 rhs=s_t[:], start=first, stop=last,
                                 skip_group_check=True)
                if last:
                    nc.vector.tensor_tensor(
                        out=aggT[:, blk * P:(blk + 1) * P],
                        in0=aggT[:, blk * P:(blk + 1) * P],
                        in1=pa[:], op=mybir.AluOpType.add)
                    del seg_psum[si]

        # ---- phase N: node MLP + residual + LayerNorm ----
        for j in range(NBLK):
            hT_t = np_.tile([P, P], BF16, tag="hT")
            nc.sync.dma_start(out=hT_t[:], in_=hTp_d[:, j * P:(j + 1) * P])
            h_t = np_.tile([P, P], F32, tag="hn")
            nc.sync.dma_start(out=h_t[:], in_=hp_d[j * P:(j + 1) * P, :])
            aggb = np_.tile([P, P], BF16, tag="aggb")
            nc.scalar.copy(out=aggb[:], in_=aggT[:, j * P:(j + 1) * P])

            pn = psn.tile([P, P], F32, space="PSUM", tag="pno")
            nc.tensor.matmul(out=pn[:], lhsT=cst["W1na"], rhs=hT_t[:],
                             start=True, stop=False, skip_group_check=True)
            nc.tensor.matmul(out=pn[:], lhsT=cst["W1nb"], rhs=aggb[:],
                             start=False, stop=True, skip_group_check=True)
            hidn = np_.tile([P, P], BF16, tag="hidn")
            nc.scalar.activation(out=hidn[:], in_=pn[:],
                                 func=mybir.ActivationFunctionType.Silu,
                                 bias=cst["b1n"])

            po = psn.tile([P, P], F32, space="PSUM", tag="pno")
            opener(po[:], rhs_ap=cst["b2nr"], lhs_ap=cst["ocol"])
            nc.tensor.matmul(out=po[:], lhsT=hidn[:], rhs=cst["W2n"],
                             start=False, stop=True, skip_group_check=True)

            x = np_.tile([P, P], F32, tag="x")
            nc.vector.tensor_tensor(out=x[:], in0=po[:], in1=h_t[:],
                                    op=mybir.AluOpType.add)
            st = np_.tile([P, 4], F32, tag="st")
            nc.vector.tensor_reduce(out=st[:, 0:1], in_=x[:],
                                    axis=mybir.AxisListType.X,
                                    op=mybir.AluOpType.add)
            nc.scalar.activation(out=st[:, 1:2], in_=st[:, 0:1],
                                 func=mybir.ActivationFunctionType.Copy,
                                 scale=1.0 / H)
            xm = np_.tile([P, P], F32, tag="xm")
            nc.vector.tensor_scalar(out=xm[:], in0=x[:],
                                    scalar1=st[:, 1:2], scalar2=None,
                                    op0=mybir.AluOpType.subtract)
            sq = np_.tile([P, P], F32, tag="sq")
            nc.scalar.activation(out=sq[:], in_=xm[:],
                                 func=mybir.ActivationFunctionType.Square,
                                 accum_out=st[:, 2:3])
            nc.scalar.activation(out=st[:, 3:4], in_=st[:, 2:3],
                                 func=mybir.ActivationFunctionType.Sqrt,
                                 scale=1.0 / H, bias=cst["epsc"])
            rs = np_.tile([P, 1], F32, tag="rs")
            nc.vector.reciprocal(out=rs[:], in_=st[:, 3:4])
            y = np_.tile([P, P], F32, tag="y")
            nc.vector.scalar_tensor_tensor(
                out=y[:], in0=xm[:], scalar=rs[:, 0:1], in1=cst["lng"],
                op0=mybir.AluOpType.mult, op1=mybir.AluOpType.mult)
            yo = np_.tile([P, P], F32, tag="yo")
            nc.vector.tensor_tensor(out=yo[:], in0=y[:], in1=cst["lnb"],
                                    op=mybir.AluOpType.add)
            nc.sync.dma_start(out=out_d[j * P:(j + 1) * P, :], in_=yo[:])


def _prepare(cfg, h, edge_index, edge_attr, edge_sh,
             W1e, b1e, W2e, b2e, W1n, b1n, W2n, b2n, ln_g, ln_b,
             loop=1):
    meta, per_core, hbf = _prep(cfg, h, edge_index, edge_attr, edge_sh)
    N, NPC = cfg["N"], cfg["NPC"]

    cpack = np.zeros((P, CPACK_W), np.float32)
    cpack[:, 0:128] = np.tile(np.asarray(ln_g, np.float32).reshape(1, -1), (P, 1))
    cpack[:, 128:256] = np.tile(np.asarray(ln_b, np.float32).reshape(1, -1), (P, 1))
    cpack[:, 256] = np.asarray(b1e, np.float32)
    cpack[:, 257] = np.asarray(b1n, np.float32)
    cpack[:, 258] = EPS

    cpbf = np.zeros((P, CPBF_W), np.float32)
    cpbf[:, 0:128] = np.tile(np.arange(P, dtype=np.float32)[None, :], (P, 1))
    cpbf[:, 128:256] = np.asarray(W2e, np.float32)
    W1n32 = np.asarray(W1n, np.float32)
    cpbf[:, 256:384] = W1n32[:H]
    cpbf[:, 384:512] = W1n32[H:]
    cpbf[:, 512:640] = np.asarray(W2n, np.float32)
    cpbf[0:TRUNC, 640:768] = np.asarray(W1e, np.float32)
    # zrow at 768:1280 stays zero
    cpbf[0, 1280:1792] = np.tile(np.asarray(b2e, np.float32), GRP)
    cpbf[0, 1792:1920] = np.asarray(b2n, np.float32)
    cpbf[0, 1920:2048] = 1.0
    weights = dict(cpack=cpack, cpbf=_bf(cpbf))

    nc = _build(cfg, meta, weights, loop=loop)
    nc.compile()

    in_maps = []
    for c in range(NCORES):
        m = dict(hbf=hbf, **per_core[c], **weights)
        in_maps.append(m)
    return nc, in_maps


def _run(cfg, h, edge_index, edge_attr, edge_sh,
         W1e, b1e, W2e, b2e, W1n, b1n, W2n, b2n, ln_g, ln_b, trace=False):
    nc, in_maps = _prepare(cfg, h, edge_index, edge_attr, edge_sh,
                           W1e, b1e, W2e, b2e, W1n, b1n, W2n, b2n, ln_g, ln_b)
    res = run_bass_kernel_spmd(nc, in_maps, list(range(NCORES)), trace=trace)
    out = np.concatenate([res.results[c]["out"] for c in range(NCORES)], axis=0)
    return out[:cfg["N"]], res


def kernel(h, edge_index, edge_attr, edge_sh,
           W1e, b1e, W2e, b2e, W1n, b1n, W2n, b2n, ln_g, ln_b):
    cfg = _full_cfg(N=h.shape[0], E=edge_index.shape[1])
    out, _ = _run(cfg, h, edge_index, edge_attr, edge_sh,
                  W1e, b1e, W2e, b2e, W1n, b1n, W2n, b2n, ln_g, ln_b)
    return out


# revision 7
# speedup vs baseline: 1.8271x; 1.3189x over previous
"""CrystalConvLayer (GNN message passing) on 8 Trainium2 NeuronCores.

Strategy (node-partitioned, edge-parallel, zero collectives):
  - Core c owns nodes [c*12544, (c+1)*12544). Edges are routed to the core
    owning their dst node; h is replicated to every core for the src gather.
  - Edge MLP runs as feature-major matmul chains on PE in bf16 (1 cycle/row
    vs 4 for fp32); PSUM accumulation stays fp32.
  - h[src] gather uses gpsimd dma_gather (int16 indices) on a bf16 copy of
    h. idx16 only covers 32k rows, so edges are bucketed by src range
    (4 buckets of 25000 rows).
  - segment_sum(messages, dst) is a matmul against a bf16 one-hot selection
    matrix S[e, n] = (dst_local[e] == n), accumulated in fp32 PSUM.
  - Stripe-major schedule: node blocks are processed in stripes of 8; each
    stripe owns one [128, 8*128] PSUM tile which accumulates scatter matmuls
    from all 4 src buckets, so no SBUF aggregate or DVE merge adds exist.
    The node MLP + residual + LayerNorm for a stripe's blocks run right
    after the stripe's scatter completes, overlapping the next stripe's
    edge work.
  - DMAs are batched (16 edge tiles per einT load, 4 node blocks per
    h/hT/out transfer) to keep the sync-engine sequencer off the critical
    path (~600ns per issued DMA).
  - SPMD: one program for all 8 cores; per-(bucket,block) segment sizes are
    padded to the max across cores so the instruction stream is uniform.

PE matmuls are limited to ONE sync-wait by walrus codegen, so every PSUM
accumulation group is opened by a rank-1 matmul on constant operands (zeros
or the bias row) which absorbs the PSUM WAR wait; data matmuls then carry at
most one wait each.
"""

import sys

if "/opt/trn_rl_repo" not in sys.path:
    sys.path.insert(0, "/opt/trn_rl_repo")

import numpy as np
import ml_dtypes
from contextlib import ExitStack

from concourse import bacc, bass, mybir, tile
from concourse.bass_utils import run_bass_kernel_spmd

F32 = mybir.dt.float32
BF16 = mybir.dt.bfloat16
I16 = mybir.dt.int16
NPBF = ml_dtypes.bfloat16

P = 128
H = 128
TRUNC = 50
EPS = 1e-5
NCORES = 8
GRP = 4          # edge tiles per mm1/psum group (free dim 512)
CH_TILES = 8     # edge tiles per dma_gather call; >=1536 idxs crashes ucode
EB = 4           # groups per einT DMA batch (16 tiles)
SW = 8           # stripe width in node blocks
NQ = 4           # node blocks per node-phase quad


def _full_cfg(N, E):
    npc = -(-N // (NCORES * P)) * P          # padded nodes per core
    return dict(
        N=N, E=E,
        NPC=npc,
        NBLK=npc // P,
        BUCKET=25000 if N > 25000 else -(-N // 4),
    )


def _bf(x):
    return np.ascontiguousarray(np.asarray(x, np.float32).astype(NPBF))


def _prep(cfg, h, edge_index, edge_attr, edge_sh):
    """Host-side sharding: route/sort/pad edges, build gather indices.

    Tile order: stripe (SW node blocks) -> src bucket -> block. Each
    (stripe, bucket) cell is padded to a GRP multiple so mm/mult groups
    never straddle a cell; gather chunks are runs of <=CH_TILES tiles
    within a cell. The last cell is further padded so NT % (GRP*EB) == 0.
    """
    N, NPC, NBLK, BUCKET = cfg["N"], cfg["NPC"], cfg["NBLK"], cfg["BUCKET"]
    NBUCK = -(-N // BUCKET)
    src = np.asarray(edge_index[0]).astype(np.int64)
    dst = np.asarray(edge_index[1]).astype(np.int64)
    ein = np.ascontiguousarray(
        np.concatenate([edge_attr, edge_sh], axis=1)[:, :TRUNC].astype(np.float32)
    )
    core_of = dst // NPC

    per = []
    cnt = np.zeros((NCORES, NBUCK, NBLK), np.int64)
    for c in range(NCORES):
        sel = np.nonzero(core_of == c)[0]
        dl = dst[sel] - c * NPC
        bk = src[sel] // BUCKET
        bl = dl // P
        o = np.lexsort((bl, bk))          # edges sorted by (bucket, blk)
        sel, dl, bk, bl = sel[o], dl[o], bk[o], bl[o]
        np.add.at(cnt[c], (bk, bl), 1)
        per.append((sel, dl))
    # edge ranges per (bucket, blk) in the per-core sorted stream
    starts = np.zeros((NCORES, NBUCK, NBLK), np.int64)
    for c in range(NCORES):
        flat = cnt[c].reshape(-1)
        starts[c] = np.concatenate([[0], np.cumsum(flat)[:-1]]).reshape(NBUCK, NBLK)

    T = -(-cnt.max(axis=0) // P)             # [NBUCK, NBLK] tiles per segment

    NSTR = -(-NBLK // SW)
    # build tile order: stripe -> bucket -> blk; collect cells and segments
    tiles = []        # per tile: (bucket, blk, idx_within_segment)
    segs_of_blk = {}  # blk -> list of global tile ids in emission order
    cells = []        # (bucket, tile_start, ntiles) gather cells
    for s in range(NSTR):
        b0, b1 = s * SW, min((s + 1) * SW, NBLK)
        for b in range(NBUCK):
            c0 = len(tiles)
            for k in range(b0, b1):
                for i in range(int(T[b, k])):
                    tiles.append((b, k, i))
                    segs_of_blk.setdefault(k, []).append(len(tiles) - 1)
            pad = (-(len(tiles) - c0)) % GRP
            if s == NSTR - 1 and b == NBUCK - 1:
                pad = (-(len(tiles) - c0) - (c0 % (GRP * EB))) % (GRP * EB)
            for _ in range(pad):
                tiles.append((b, None, None))
            cells.append((b, c0, len(tiles) - c0))
    NT = len(tiles)
    assert NT % (GRP * EB) == 0
    EP = NT * P

    chunks = []                              # (bucket, tile_start, ntiles)
    for b, c0, n in cells:
        t = 0
        while t < n:
            m = min(CH_TILES, n - t)
            chunks.append((b, c0 + t, m))
            t += m

    # per-tile scatter routing: (blk, first, last) or None for pad tiles
    troute = [None] * NT
    for k, tl in segs_of_blk.items():
        for j, t in enumerate(tl):
            troute[t] = (k, j == 0, j == len(tl) - 1)
    # tile id after which each stripe's scatter is fully closed
    stripe_done = []
    stop_tiles = set()
    for s in range(NSTR):
        b0, b1 = s * SW, min((s + 1) * SW, NBLK)
        last = max(segs_of_blk[k][-1] for k in range(b0, b1) if k in segs_of_blk)
        stripe_done.append(last)
        # one stop per 4-block PSUM bank (512 fp32 columns)
        for bk0 in range(b0, b1, 4):
            blks = [k for k in range(bk0, min(bk0 + 4, b1)) if k in segs_of_blk]
            if blks:
                stop_tiles.add(max(segs_of_blk[k][-1] for k in blks))

    hpad = np.zeros((NCORES * NPC, H), np.float32)
    hpad[:N] = np.asarray(h, np.float32)

    per_core = []
    for c in range(NCORES):
        sel, dl = per[c]
        ein_pad = np.zeros((EP, TRUNC), np.float32)
        gi = np.zeros(EP, np.int16)
        df = np.full(EP, 300.0, np.float32)
        for t, (b, k, i) in enumerate(tiles):
            if k is None:
                continue
            e0 = int(starts[c, b, k])
            n = int(cnt[c, b, k])
            lo, hi = i * P, min((i + 1) * P, n)
            if lo >= n:
                continue
            m = hi - lo
            s_ = sel[e0 + lo:e0 + hi]
            off = t * P
            ein_pad[off:off + m] = ein[s_]
            gi[off:off + m] = (src[s_] - b * BUCKET).astype(np.int16)
            df[off:off + m] = (dl[e0 + lo:e0 + hi] % P).astype(np.float32)
        giw = np.tile(gi.reshape(EP // 16, 16).T, (8, 1))     # [128, EP/16]
        dfw = df.reshape(NT, P).T.copy()                      # [128, NT]
        hs = hpad[c * NPC:(c + 1) * NPC]
        per_core.append(dict(
            einT=_bf(ein_pad.T),
            gidx=np.ascontiguousarray(giw),
            dstf=np.ascontiguousarray(dfw),
            hTp=_bf(hs.T),
            hp=np.ascontiguousarray(hs),
        ))

    hbf = _bf(np.asarray(h, np.float32))
    meta = dict(NBUCK=NBUCK, NT=NT, EP=EP, NSTR=NSTR, chunks=chunks,
                troute=troute, stripe_done=stripe_done, stop_tiles=stop_tiles)
    return meta, per_core, hbf


def _build(cfg, meta, weights, loop=1):
    """Emit the SPMD Bass program (see module docstring for wait discipline)."""
    N, NPC, NBLK, BUCKET = cfg["N"], cfg["NPC"], cfg["NBLK"], cfg["BUCKET"]
    NBUCK, NT, EP = meta["NBUCK"], meta["NT"], meta["EP"]

    nc = bacc.Bacc("TRN2", target_bir_lowering=False, debug=False,
                   num_devices=NCORES)

    hbf_d = nc.dram_tensor("hbf", [N, H], BF16, kind="ExternalInput")
    einT_d = nc.dram_tensor("einT", [TRUNC, EP], BF16, kind="ExternalInput")
    gidx_d = nc.dram_tensor("gidx", [P, EP // 16], I16, kind="ExternalInput")
    dstf_d = nc.dram_tensor("dstf", [P, NT], F32, kind="ExternalInput")
    hTp_d = nc.dram_tensor("hTp", [P, NPC], BF16, kind="ExternalInput")
    hp_d = nc.dram_tensor("hp", [NPC, H], F32, kind="ExternalInput")
    cst_d = {k: nc.dram_tensor(k, list(v.shape),
                               BF16 if v.dtype == NPBF else F32,
                               kind="ExternalInput")
             for k, v in weights.items()}
    out_d = nc.dram_tensor("out", [NPC, H], F32, kind="ExternalOutput")

    with tile.TileContext(nc) as tc, ExitStack() as ctx:
        cp = ctx.enter_context(tc.tile_pool(name="cst", bufs=1))
        bigp = ctx.enter_context(tc.tile_pool(name="big", bufs=1))
        gp = ctx.enter_context(tc.tile_pool(name="gch", bufs=6))
        ep = ctx.enter_context(tc.tile_pool(name="edge", bufs=3))
        sp_ = ctx.enter_context(tc.tile_pool(name="spool", bufs=4))
        np_ = ctx.enter_context(tc.tile_pool(name="node", bufs=5))
        ps1 = ctx.enter_context(tc.tile_pool(name="ps1", bufs=2, space="PSUM"))
        psw = ctx.enter_context(tc.tile_pool(name="psw", bufs=2, space="PSUM"))
        psS = ctx.enter_context(tc.tile_pool(name="psS", bufs=1, space="PSUM"))
        psn = ctx.enter_context(tc.tile_pool(name="psn", bufs=2, space="PSUM"))

        # ---- constants: two DMAs (f32 + bf16 packs), sliced views ----
        cpk = cp.tile([P, 512], F32, tag="cpack")
        nc.sync.dma_start(out=cpk[:], in_=cst_d["cpack"][:])
        cpb = cp.tile([P, 3072], BF16, tag="cpbf")
        nc.sync.dma_start(out=cpb[:], in_=cst_d["cpbf"][:])
        cst = dict(
            lng=cpk[:, 0:128], lnb=cpk[:, 128:256],
            b1e=cpk[:, 256:257], b1n=cpk[:, 257:258], epsc=cpk[:, 258:259],
            iota=cpb[:, 0:128], W2e=cpb[:, 512:640],
            W1na=cpb[:, 640:768], W1nb=cpb[:, 768:896], W2n=cpb[:, 896:1024],
            W1e=cpb[0:TRUNC, 1024:1152],
            zrow=cpb[0:1, 1152:1664], zcol=cpb[0:1, 1152:1280],
            b2e4=cpb[0:1, 1664:2176], b2n4=cpb[0:1, 2432:2944],
            ocol=cpb[0:1, 2304:2432],
        )
        gidx_sb = bigp.tile([P, EP // 16], I16)
        nc.sync.dma_start(out=gidx_sb[:], in_=gidx_d[:])
        dstf_sb = bigp.tile([P, NT], F32)
        nc.sync.dma_start(out=dstf_sb[:], in_=dstf_d[:])

        def opener(out_ap, rhs_ap=None, lhs_ap=None):
            nc.tensor.matmul(
                out=out_ap,
                lhsT=(lhs_ap if lhs_ap is not None else cst["zcol"]),
                rhs=(rhs_ap if rhs_ap is not None else
                     cst["zrow"][:, :out_ap.shape[-1]]),
                start=True, stop=False, skip_group_check=True)

        def emit_body():
            _emit(nc, tc, cfg, meta, cst, gidx_sb, dstf_sb, opener,
                  hbf_d, einT_d, hTp_d, hp_d, out_d,
                  gp, ep, sp_, np_, ps1, psw, psS, psn)

        if loop > 1:
            with tc.For_i(0, loop, 1):
                emit_body()
        else:
            emit_body()

    return nc


def _emit(nc, tc, cfg, meta, cst, gidx_sb, dstf_sb, opener,
          hbf_d, einT_d, hTp_d, hp_d, out_d,
          gp, ep, sp_, np_, ps1, psw, psS, psn):
    N, NPC, NBLK, BUCKET = cfg["N"], cfg["NPC"], cfg["NBLK"], cfg["BUCKET"]
    NBUCK, NT, EP, NSTR = meta["NBUCK"], meta["NT"], meta["EP"], meta["NSTR"]
    chunks, troute, stripe_done = (meta["chunks"], meta["troute"],
                                   meta["stripe_done"])
    stop_tiles = meta["stop_tiles"]

    def open_banks(agg_ps, nblk):
        for c0 in range(0, nblk * P, 512):
            opener(agg_ps[:, c0:c0 + 512])

    def node_quad(j0, nb, agg_ps, poff):
        """Node MLP + residual + LayerNorm for blocks j0..j0+nb-1.
        agg_ps[:, poff*P : (poff+nb)*P] holds their aggregates (fp32 PSUM)."""
        W = nb * P
        aggb = np_.tile([P, NQ * P], BF16, tag="aggb")
        nc.scalar.copy(out=aggb[:, :W], in_=agg_ps[:, poff * P:(poff + nb) * P])
        hT_t = np_.tile([P, NQ * P], BF16, tag="hT")
        nc.sync.dma_start(out=hT_t[:, :W],
                          in_=hTp_d[:, j0 * P:(j0 + nb) * P])
        h_t = np_.tile([P, NQ, P], F32, tag="hn")
        nc.sync.dma_start(
            out=h_t[:, :nb, :],
            in_=hp_d[j0 * P:(j0 + nb) * P, :].rearrange("(a p) h -> p a h", p=P))

        pn = psn.tile([P, NQ * P], F32, space="PSUM", tag="pno")
        for i in range(nb):
            nc.tensor.matmul(out=pn[:, i * P:(i + 1) * P], lhsT=cst["W1na"],
                             rhs=hT_t[:, i * P:(i + 1) * P],
                             start=True, stop=False, skip_group_check=True)
            nc.tensor.matmul(out=pn[:, i * P:(i + 1) * P], lhsT=cst["W1nb"],
                             rhs=aggb[:, i * P:(i + 1) * P],
                             start=False, stop=True, skip_group_check=True)
        hidn = np_.tile([P, NQ * P], BF16, tag="hidn")
        nc.scalar.activation(out=hidn[:, :W], in_=pn[:, :W],
                             func=mybir.ActivationFunctionType.Silu,
                             bias=cst["b1n"])

        po = psn.tile([P, NQ * P], F32, space="PSUM", tag="pno")
        opener(po[:, :W], rhs_ap=cst["b2n4"][:, :W], lhs_ap=cst["ocol"])
        for i in range(nb):
            nc.tensor.matmul(out=po[:, i * P:(i + 1) * P],
                             lhsT=hidn[:, i * P:(i + 1) * P], rhs=cst["W2n"],
                             start=False, stop=True, skip_group_check=True)

        yo = np_.tile([P, NQ, P], F32, tag="yo")
        for i in range(nb):
            x = np_.tile([P, P], F32, tag="x")
            nc.vector.tensor_tensor(out=x[:], in0=po[:, i * P:(i + 1) * P],
                                    in1=h_t[:, i, :], op=mybir.AluOpType.add)
            st = np_.tile([P, 4], F32, tag="st")
            nc.vector.tensor_reduce(out=st[:, 0:1], in_=x[:],
                                    axis=mybir.AxisListType.X,
                                    op=mybir.AluOpType.add)
            nc.scalar.activation(out=st[:, 1:2], in_=st[:, 0:1],
                                 func=mybir.ActivationFunctionType.Copy,
                                 scale=1.0 / H)
            xm = np_.tile([P, P], F32, tag="xm")
            nc.vector.tensor_scalar(out=xm[:], in0=x[:],
                                    scalar1=st[:, 1:2], scalar2=None,
                                    op0=mybir.AluOpType.subtract)
            sq = np_.tile([P, P], F32, tag="sq")
            nc.scalar.activation(out=sq[:], in_=xm[:],
                                 func=mybir.ActivationFunctionType.Square,
                                 accum_out=st[:, 2:3])
            nc.scalar.activation(out=st[:, 3:4], in_=st[:, 2:3],
                                 func=mybir.ActivationFunctionType.Sqrt,
                                 scale=1.0 / H, bias=cst["epsc"])
            rs = np_.tile([P, 1], F32, tag="rs")
            nc.vector.reciprocal(out=rs[:], in_=st[:, 3:4])
            y = np_.tile([P, P], F32, tag="y")
            nc.vector.scalar_tensor_tensor(
                out=y[:], in0=xm[:], scalar=rs[:, 0:1], in1=cst["lng"],
                op0=mybir.AluOpType.mult, op1=mybir.AluOpType.mult)
            nc.vector.tensor_tensor(out=yo[:, i, :], in0=y[:], in1=cst["lnb"],
                                    op=mybir.AluOpType.add)
        nc.sync.dma_start(
            out=out_d[j0 * P:(j0 + nb) * P, :].rearrange("(a p) h -> p a h", p=P),
            in_=yo[:, :nb, :])

    # ---- main edge loop, stripes interleaved ----
    gch = None
    gch_t0 = 0
    ci = 0
    ein_t = None
    stripe = 0                      # stripe whose psum tile is being filled
    agg_ps = psS.tile([P, SW * P], F32, space="PSUM", tag="agg")
    open_banks(agg_ps, min(SW, NBLK))
    for g in range(NT // GRP):
        t0 = g * GRP
        if ci < len(chunks) and chunks[ci][1] == t0:
            b, ts, nt = chunks[ci]
            ci += 1
            gch = gp.tile([P, CH_TILES, H], BF16, tag="gch")
            gch_t0 = ts
            base = b * BUCKET
            rows = min(BUCKET, N - base)
            nc.gpsimd.dma_gather(
                out_ap=gch[:, :nt, :],
                in_ap=hbf_d[base:base + rows, :],
                idxs_ap=gidx_sb[:, ts * 8:(ts + nt) * 8],
                num_idxs=nt * P,
                num_idxs_reg=nt * P,
                elem_size=H,
            )
        if g % EB == 0:
            ein_t = ep.tile([TRUNC, EB * GRP * P], BF16, tag="ein")
            nc.sync.dma_start(
                out=ein_t[:],
                in_=einT_d[:, t0 * P:(t0 + EB * GRP) * P])
        eo = (g % EB) * GRP * P

        p1 = ps1.tile([P, GRP * P], F32, space="PSUM", tag="p1")
        nc.tensor.matmul(out=p1[:], lhsT=cst["W1e"],
                         rhs=ein_t[:, eo:eo + GRP * P],
                         start=True, stop=True, skip_group_check=True)
        hidT = ep.tile([P, GRP * P], BF16, tag="hidT")
        nc.scalar.activation(out=hidT[:], in_=p1[:],
                             func=mybir.ActivationFunctionType.Silu,
                             bias=cst["b1e"])

        pw = psw.tile([P, GRP * P], F32, space="PSUM", tag="pw")
        opener(pw[:], rhs_ap=cst["b2e4"], lhs_ap=cst["ocol"])
        for i in range(GRP):
            nc.tensor.matmul(out=pw[:, i * P:(i + 1) * P],
                             lhsT=hidT[:, i * P:(i + 1) * P],
                             rhs=cst["W2e"],
                             start=False, stop=(i == GRP - 1),
                             skip_group_check=True)

        msg = ep.tile([P, GRP * P], BF16, tag="msg")
        o = t0 - gch_t0
        gin = gch[:, o:o + GRP, :].rearrange("p a b -> p (a b)")
        nc.vector.tensor_tensor(out=msg[:], in0=pw[:], in1=gin,
                                op=mybir.AluOpType.mult)

        s4 = sp_.tile([P, GRP * P], BF16, tag="S")
        for i in range(GRP):
            nc.vector.tensor_scalar(
                out=s4[:, i * P:(i + 1) * P], in0=cst["iota"],
                scalar1=dstf_sb[:, t0 + i:t0 + i + 1], scalar2=None,
                op0=mybir.AluOpType.is_equal)

        for i in range(GRP):
            t = t0 + i
            r = troute[t]
            if r is None:               # pure padding tile
                continue
            blk, first, last = r
            poff = blk - stripe * SW
            nc.tensor.matmul(out=agg_ps[:, poff * P:(poff + 1) * P],
                             lhsT=msg[:, i * P:(i + 1) * P],
                             rhs=s4[:, i * P:(i + 1) * P],
                             start=False, stop=(t in stop_tiles),
                             skip_group_check=True)

        if stripe < NSTR and t0 + GRP - 1 >= stripe_done[stripe]:
            # stripe's scatter closed: run its node phase, open next stripe
            b0 = stripe * SW
            nb_total = min(SW, NBLK - b0)
            q = 0
            while q < nb_total:
                nb = min(NQ, nb_total - q)
                node_quad(b0 + q, nb, agg_ps, q)
                q += nb
            stripe += 1
            if stripe < NSTR:
                agg_ps = psS.tile([P, SW * P], F32, space="PSUM", tag="agg")
                open_banks(agg_ps, min(SW, NBLK - stripe * SW))


def _prepare(cfg, h, edge_index, edge_attr, edge_sh,
             W1e, b1e, W2e, b2e, W1n, b1n, W2n, b2n, ln_g, ln_b,
             loop=1):
    meta, per_core, hbf = _prep(cfg, h, edge_index, edge_attr, edge_sh)

    cpack = np.zeros((P, 512), np.float32)
    cpack[:, 0:128] = np.tile(np.asarray(ln_g, np.float32).reshape(1, -1), (P, 1))
    cpack[:, 128:256] = np.tile(np.asarray(ln_b, np.float32).reshape(1, -1), (P, 1))
    cpack[:, 256] = np.asarray(b1e, np.float32)
    cpack[:, 257] = np.asarray(b1n, np.float32)
    cpack[:, 258] = EPS

    cpbf = np.zeros((P, 3072), np.float32)
    cpbf[:, 0:128] = np.tile(np.arange(P, dtype=np.float32)[None, :], (P, 1))
    cpbf[:, 512:640] = np.asarray(W2e, np.float32)
    W1n32 = np.asarray(W1n, np.float32)
    cpbf[:, 640:768] = W1n32[:H]
    cpbf[:, 768:896] = W1n32[H:]
    cpbf[:, 896:1024] = np.asarray(W2n, np.float32)
    cpbf[0:TRUNC, 1024:1152] = np.asarray(W1e, np.float32)
    # zrow at 1152:1664 stays zero
    cpbf[0, 1664:2176] = np.tile(np.asarray(b2e, np.float32), GRP)
    cpbf[0, 2432:2944] = np.tile(np.asarray(b2n, np.float32), NQ)
    cpbf[0, 2304:2432] = 1.0
    weights = dict(cpack=cpack, cpbf=_bf(cpbf))

    nc = _build(cfg, meta, weights, loop=loop)
    nc.compile()

    in_maps = []
    for c in range(NCORES):
        m = dict(hbf=hbf, **per_core[c], **weights)
        in_maps.append(m)
    return nc, in_maps


def _run(cfg, h, edge_index, edge_attr, edge_sh,
         W1e, b1e, W2e, b2e, W1n, b1n, W2n, b2n, ln_g, ln_b, trace=False):
    nc, in_maps = _prepare(cfg, h, edge_index, edge_attr, edge_sh,
                           W1e, b1e, W2e, b2e, W1n, b1n, W2n, b2n, ln_g, ln_b)
    res = run_bass_kernel_spmd(nc, in_maps, list(range(NCORES)), trace=trace)
    out = np.concatenate([res.results[c]["out"] for c in range(NCORES)], axis=0)
    return out[:cfg["N"]], res


def kernel(h, edge_index, edge_attr, edge_sh,
           W1e, b1e, W2e, b2e, W1n, b1n, W2n, b2n, ln_g, ln_b):
    cfg = _full_cfg(N=h.shape[0], E=edge_index.shape[1])
    out, _ = _run(cfg, h, edge_index, edge_attr, edge_sh,
                  W1e, b1e, W2e, b2e, W1n, b1n, W2n, b2n, ln_g, ln_b)
    return out


# revision 8
# speedup vs baseline: 1.8791x; 1.0284x over previous
"""CrystalConvLayer (GNN message passing) on 8 Trainium2 NeuronCores.

Strategy (node-partitioned, edge-parallel, zero collectives):
  - Core c owns nodes [c*12544, (c+1)*12544). Edges are routed to the core
    owning their dst node; h is replicated to every core for the src gather.
  - Edge MLP runs as feature-major matmul chains on PE in bf16 (1 cycle/row
    vs 4 for fp32); PSUM accumulation stays fp32.
  - h[src] gather uses gpsimd dma_gather (int16 indices) on a bf16 copy of
    h. idx16 only covers 32k rows, so edges are bucketed by src range
    (4 buckets of 25000 rows).
  - segment_sum(messages, dst) is a matmul against a bf16 one-hot selection
    matrix S[e, n] = (dst_local[e] == n), accumulated in fp32 PSUM.
  - Stripe-major schedule: node blocks are processed in stripes of 8; each
    stripe owns one [128, 8*128] PSUM tile which accumulates scatter matmuls
    from all 4 src buckets, so no SBUF aggregate or DVE merge adds exist.
    The node MLP + residual + LayerNorm for a stripe's blocks run right
    after the stripe's scatter completes, overlapping the next stripe's
    edge work.
  - DMAs are batched (16 edge tiles per einT load, 4 node blocks per
    h/hT/out transfer) to keep the sync-engine sequencer off the critical
    path (~600ns per issued DMA).
  - SPMD: one program for all 8 cores; per-(bucket,block) segment sizes are
    padded to the max across cores so the instruction stream is uniform.

PE matmuls are limited to ONE sync-wait by walrus codegen, so every PSUM
accumulation group is opened by a rank-1 matmul on constant operands (zeros
or the bias row) which absorbs the PSUM WAR wait; data matmuls then carry at
most one wait each.
"""

import sys

if "/opt/trn_rl_repo" not in sys.path:
    sys.path.insert(0, "/opt/trn_rl_repo")

import numpy as np
import ml_dtypes
from contextlib import ExitStack

from concourse import bacc, bass, mybir, tile
from concourse.bass_utils import run_bass_kernel_spmd

F32 = mybir.dt.float32
BF16 = mybir.dt.bfloat16
I16 = mybir.dt.int16
NPBF = ml_dtypes.bfloat16

P = 128
H = 128
TRUNC = 50
EPS = 1e-5
NCORES = 8
GRP = 4          # edge tiles per mm1/psum group (free dim 512)
CH_TILES = 8     # edge tiles per dma_gather call; >=1536 idxs crashes ucode
EB = 4           # groups per einT DMA batch (16 tiles)
SW = 8           # stripe width in node blocks
NQ = 4           # node blocks per node-phase quad


def _full_cfg(N, E):
    npc = -(-N // (NCORES * P)) * P          # padded nodes per core
    return dict(
        N=N, E=E,
        NPC=npc,
        NBLK=npc // P,
        BUCKET=25000 if N > 25000 else -(-N // 4),
    )


def _bf(x):
    return np.ascontiguousarray(np.asarray(x, np.float32).astype(NPBF))


def _prep(cfg, h, edge_index, edge_attr, edge_sh):
    """Host-side sharding: route/sort/pad edges, build gather indices.

    Tile order: stripe (SW node blocks) -> src bucket -> block. Each
    (stripe, bucket) cell is padded to a GRP multiple so mm/mult groups
    never straddle a cell; gather chunks are runs of <=CH_TILES tiles
    within a cell. The last cell is further padded so NT % (GRP*EB) == 0.
    """
    N, NPC, NBLK, BUCKET = cfg["N"], cfg["NPC"], cfg["NBLK"], cfg["BUCKET"]
    NBUCK = -(-N // BUCKET)
    src = np.asarray(edge_index[0]).astype(np.int64)
    dst = np.asarray(edge_index[1]).astype(np.int64)
    ein = np.ascontiguousarray(
        np.concatenate([edge_attr, edge_sh], axis=1)[:, :TRUNC].astype(np.float32)
    )
    core_of = dst // NPC

    per = []
    cnt = np.zeros((NCORES, NBUCK, NBLK), np.int64)
    for c in range(NCORES):
        sel = np.nonzero(core_of == c)[0]
        dl = dst[sel] - c * NPC
        bk = src[sel] // BUCKET
        bl = dl // P
        o = np.lexsort((bl, bk))          # edges sorted by (bucket, blk)
        sel, dl, bk, bl = sel[o], dl[o], bk[o], bl[o]
        np.add.at(cnt[c], (bk, bl), 1)
        per.append((sel, dl))
    # edge ranges per (bucket, blk) in the per-core sorted stream
    starts = np.zeros((NCORES, NBUCK, NBLK), np.int64)
    for c in range(NCORES):
        flat = cnt[c].reshape(-1)
        starts[c] = np.concatenate([[0], np.cumsum(flat)[:-1]]).reshape(NBUCK, NBLK)

    T = -(-cnt.max(axis=0) // P)             # [NBUCK, NBLK] tiles per segment

    NSTR = -(-NBLK // SW)
    # build tile order: stripe -> bucket -> blk; collect cells and segments
    tiles = []        # per tile: (bucket, blk, idx_within_segment)
    segs_of_blk = {}  # blk -> list of global tile ids in emission order
    cells = []        # (bucket, tile_start, ntiles) gather cells
    for s in range(NSTR):
        b0, b1 = s * SW, min((s + 1) * SW, NBLK)
        for b in range(NBUCK):
            c0 = len(tiles)
            for k in range(b0, b1):
                for i in range(int(T[b, k])):
                    tiles.append((b, k, i))
                    segs_of_blk.setdefault(k, []).append(len(tiles) - 1)
            pad = (-(len(tiles) - c0)) % GRP
            if s == NSTR - 1 and b == NBUCK - 1:
                pad = (-(len(tiles) - c0) - (c0 % (GRP * EB))) % (GRP * EB)
            for _ in range(pad):
                tiles.append((b, None, None))
            cells.append((b, c0, len(tiles) - c0))
    NT = len(tiles)
    assert NT % (GRP * EB) == 0
    EP = NT * P

    chunks = []                              # (bucket, tile_start, ntiles)
    for b, c0, n in cells:
        t = 0
        while t < n:
            m = min(CH_TILES, n - t)
            chunks.append((b, c0 + t, m))
            t += m

    # per-tile scatter routing: (blk, first, last) or None for pad tiles
    troute = [None] * NT
    for k, tl in segs_of_blk.items():
        for j, t in enumerate(tl):
            troute[t] = (k, j == 0, j == len(tl) - 1)
    # tile id after which each stripe's scatter is fully closed
    stripe_done = []
    stop_tiles = set()
    for s in range(NSTR):
        b0, b1 = s * SW, min((s + 1) * SW, NBLK)
        last = max(segs_of_blk[k][-1] for k in range(b0, b1) if k in segs_of_blk)
        stripe_done.append(last)
        # one stop per 4-block PSUM bank (512 fp32 columns)
        for bk0 in range(b0, b1, 4):
            blks = [k for k in range(bk0, min(bk0 + 4, b1)) if k in segs_of_blk]
            if blks:
                stop_tiles.add(max(segs_of_blk[k][-1] for k in blks))

    hpad = np.zeros((NCORES * NPC, H), np.float32)
    hpad[:N] = np.asarray(h, np.float32)

    per_core = []
    for c in range(NCORES):
        sel, dl = per[c]
        ein_pad = np.zeros((EP, TRUNC), np.float32)
        gi = np.zeros(EP, np.int16)
        df = np.full(EP, 300.0, np.float32)
        for t, (b, k, i) in enumerate(tiles):
            if k is None:
                continue
            e0 = int(starts[c, b, k])
            n = int(cnt[c, b, k])
            lo, hi = i * P, min((i + 1) * P, n)
            if lo >= n:
                continue
            m = hi - lo
            s_ = sel[e0 + lo:e0 + hi]
            off = t * P
            ein_pad[off:off + m] = ein[s_]
            gi[off:off + m] = (src[s_] - b * BUCKET).astype(np.int16)
            df[off:off + m] = (dl[e0 + lo:e0 + hi] % P).astype(np.float32)
        giw = np.tile(gi.reshape(EP // 16, 16).T, (8, 1))     # [128, EP/16]
        dfw = df.reshape(NT, P).T.copy()                      # [128, NT]
        hs = hpad[c * NPC:(c + 1) * NPC]
        per_core.append(dict(
            einT=_bf(ein_pad.T),
            gidx=np.ascontiguousarray(giw),
            dstf=np.ascontiguousarray(dfw),
            hTp=_bf(hs.T),
            hp=np.ascontiguousarray(hs),
        ))

    hbf = _bf(np.asarray(h, np.float32))
    meta = dict(NBUCK=NBUCK, NT=NT, EP=EP, NSTR=NSTR, chunks=chunks,
                troute=troute, stripe_done=stripe_done, stop_tiles=stop_tiles)
    return meta, per_core, hbf


def _build(cfg, meta, weights, loop=1):
    """Emit the SPMD Bass program (see module docstring for wait discipline)."""
    N, NPC, NBLK, BUCKET = cfg["N"], cfg["NPC"], cfg["NBLK"], cfg["BUCKET"]
    NBUCK, NT, EP = meta["NBUCK"], meta["NT"], meta["EP"]

    nc = bacc.Bacc("TRN2", target_bir_lowering=False, debug=False,
                   num_devices=NCORES)

    hbf_d = nc.dram_tensor("hbf", [N, H], BF16, kind="ExternalInput")
    einT_d = nc.dram_tensor("einT", [TRUNC, EP], BF16, kind="ExternalInput")
    gidx_d = nc.dram_tensor("gidx", [P, EP // 16], I16, kind="ExternalInput")
    dstf_d = nc.dram_tensor("dstf", [P, NT], F32, kind="ExternalInput")
    hTp_d = nc.dram_tensor("hTp", [P, NPC], BF16, kind="ExternalInput")
    hp_d = nc.dram_tensor("hp", [NPC, H], F32, kind="ExternalInput")
    cst_d = {k: nc.dram_tensor(k, list(v.shape),
                               BF16 if v.dtype == NPBF else F32,
                               kind="ExternalInput")
             for k, v in weights.items()}
    out_d = nc.dram_tensor("out", [NPC, H], F32, kind="ExternalOutput")

    with tile.TileContext(nc) as tc, ExitStack() as ctx:
        cp = ctx.enter_context(tc.tile_pool(name="cst", bufs=1))
        bigp = ctx.enter_context(tc.tile_pool(name="big", bufs=1))
        gp = ctx.enter_context(tc.tile_pool(name="gch", bufs=6))
        ep = ctx.enter_context(tc.tile_pool(name="edge", bufs=3))
        sp_ = ctx.enter_context(tc.tile_pool(name="spool", bufs=4))
        np_ = ctx.enter_context(tc.tile_pool(name="node", bufs=5))
        ps1 = ctx.enter_context(tc.tile_pool(name="ps1", bufs=2, space="PSUM"))
        psw = ctx.enter_context(tc.tile_pool(name="psw", bufs=2, space="PSUM"))
        psS = ctx.enter_context(tc.tile_pool(name="psS", bufs=1, space="PSUM"))
        psn = ctx.enter_context(tc.tile_pool(name="psn", bufs=2, space="PSUM"))

        # ---- constants: two DMAs (f32 + bf16 packs), sliced views ----
        cpk = cp.tile([P, 512], F32, tag="cpack")
        nc.sync.dma_start(out=cpk[:], in_=cst_d["cpack"][:])
        cpb = cp.tile([P, 3072], BF16, tag="cpbf")
        nc.sync.dma_start(out=cpb[:], in_=cst_d["cpbf"][:])
        cst = dict(
            lng=cpk[:, 0:128], lnb=cpk[:, 128:256],
            b1e=cpk[:, 256:257], b1n=cpk[:, 257:258], epsc=cpk[:, 258:259],
            iota=cpb[:, 0:128], W2e=cpb[:, 512:640],
            W1na=cpb[:, 640:768], W1nb=cpb[:, 768:896], W2n=cpb[:, 896:1024],
            W1e=cpb[0:TRUNC, 1024:1152],
            zrow=cpb[0:1, 1152:1664], zcol=cpb[0:1, 1152:1280],
            b2e4=cpb[0:1, 1664:2176], b2n4=cpb[0:1, 2432:2944],
            ocol=cpb[0:1, 2304:2432],
        )
        gidx_sb = bigp.tile([P, EP // 16], I16)
        nc.sync.dma_start(out=gidx_sb[:], in_=gidx_d[:])
        dstf_sb = bigp.tile([P, NT], F32)
        nc.sync.dma_start(out=dstf_sb[:], in_=dstf_d[:])

        def opener(out_ap, rhs_ap=None, lhs_ap=None):
            nc.tensor.matmul(
                out=out_ap,
                lhsT=(lhs_ap if lhs_ap is not None else cst["zcol"]),
                rhs=(rhs_ap if rhs_ap is not None else
                     cst["zrow"][:, :out_ap.shape[-1]]),
                start=True, stop=False, skip_group_check=True)

        def emit_body():
            _emit(nc, tc, cfg, meta, cst, gidx_sb, dstf_sb, opener,
                  hbf_d, einT_d, hTp_d, hp_d, out_d,
                  gp, ep, sp_, np_, ps1, psw, psS, psn)

        if loop > 1:
            with tc.For_i(0, loop, 1):
                emit_body()
        else:
            emit_body()

    return nc


def _emit(nc, tc, cfg, meta, cst, gidx_sb, dstf_sb, opener,
          hbf_d, einT_d, hTp_d, hp_d, out_d,
          gp, ep, sp_, np_, ps1, psw, psS, psn):
    N, NPC, NBLK, BUCKET = cfg["N"], cfg["NPC"], cfg["NBLK"], cfg["BUCKET"]
    NBUCK, NT, EP, NSTR = meta["NBUCK"], meta["NT"], meta["EP"], meta["NSTR"]
    chunks, troute, stripe_done = (meta["chunks"], meta["troute"],
                                   meta["stripe_done"])
    stop_tiles = meta["stop_tiles"]

    def open_banks(agg_ps, nblk):
        for c0 in range(0, nblk * P, 512):
            opener(agg_ps[:, c0:c0 + 512])

    def node_quad(j0, nb, agg_ps, poff):
        """Node MLP + residual + LayerNorm for blocks j0..j0+nb-1.
        agg_ps[:, poff*P : (poff+nb)*P] holds their aggregates (fp32 PSUM)."""
        W = nb * P
        aggb = np_.tile([P, NQ * P], BF16, tag="aggb")
        nc.scalar.copy(out=aggb[:, :W], in_=agg_ps[:, poff * P:(poff + nb) * P])
        hT_t = np_.tile([P, NQ * P], BF16, tag="hT")
        nc.sync.dma_start(out=hT_t[:, :W],
                          in_=hTp_d[:, j0 * P:(j0 + nb) * P])
        h_t = np_.tile([P, NQ, P], F32, tag="hn")
        nc.sync.dma_start(
            out=h_t[:, :nb, :],
            in_=hp_d[j0 * P:(j0 + nb) * P, :].rearrange("(a p) h -> p a h", p=P))

        pn = psn.tile([P, NQ * P], F32, space="PSUM", tag="pno")
        for i in range(nb):
            nc.tensor.matmul(out=pn[:, i * P:(i + 1) * P], lhsT=cst["W1na"],
                             rhs=hT_t[:, i * P:(i + 1) * P],
                             start=True, stop=False, skip_group_check=True)
            nc.tensor.matmul(out=pn[:, i * P:(i + 1) * P], lhsT=cst["W1nb"],
                             rhs=aggb[:, i * P:(i + 1) * P],
                             start=False, stop=True, skip_group_check=True)
        hidn = np_.tile([P, NQ * P], BF16, tag="hidn")
        nc.scalar.activation(out=hidn[:, :W], in_=pn[:, :W],
                             func=mybir.ActivationFunctionType.Silu,
                             bias=cst["b1n"])

        po = psn.tile([P, NQ * P], F32, space="PSUM", tag="pno")
        opener(po[:, :W], rhs_ap=cst["b2n4"][:, :W], lhs_ap=cst["ocol"])
        for i in range(nb):
            nc.tensor.matmul(out=po[:, i * P:(i + 1) * P],
                             lhsT=hidn[:, i * P:(i + 1) * P], rhs=cst["W2n"],
                             start=False, stop=True, skip_group_check=True)

        x = np_.tile([P, NQ, P], F32, tag="x")
        nc.vector.tensor_tensor(
            out=x[:, :nb, :],
            in0=po[:, :W].rearrange("p (a h) -> p a h", a=nb),
            in1=h_t[:, :nb, :], op=mybir.AluOpType.add)
        st = np_.tile([P, 16], F32, tag="st")
        nc.vector.tensor_reduce(out=st[:, 0:nb].unsqueeze(-1), in_=x[:, :nb, :],
                                axis=mybir.AxisListType.X,
                                op=mybir.AluOpType.add)
        nc.scalar.activation(out=st[:, 4:4 + nb], in_=st[:, 0:nb],
                             func=mybir.ActivationFunctionType.Copy,
                             scale=1.0 / H)
        xm = np_.tile([P, NQ, P], F32, tag="xm")
        y = np_.tile([P, NQ, P], F32, tag="y")
        for i in range(nb):
            nc.vector.tensor_scalar(out=xm[:, i, :], in0=x[:, i, :],
                                    scalar1=st[:, 4 + i:5 + i], scalar2=None,
                                    op0=mybir.AluOpType.subtract)
            sq = np_.tile([P, P], F32, tag="sq")
            nc.scalar.activation(out=sq[:], in_=xm[:, i, :],
                                 func=mybir.ActivationFunctionType.Square,
                                 accum_out=st[:, 8 + i:9 + i])
        nc.scalar.activation(out=st[:, 12:12 + nb], in_=st[:, 8:8 + nb],
                             func=mybir.ActivationFunctionType.Sqrt,
                             scale=1.0 / H, bias=cst["epsc"])
        rs = np_.tile([P, 4], F32, tag="rs")
        nc.vector.reciprocal(out=rs[:, :nb], in_=st[:, 12:12 + nb])
        for i in range(nb):
            nc.vector.scalar_tensor_tensor(
                out=y[:, i, :], in0=xm[:, i, :], scalar=rs[:, i:i + 1],
                in1=cst["lng"],
                op0=mybir.AluOpType.mult, op1=mybir.AluOpType.mult)
        yo = np_.tile([P, NQ, P], F32, tag="yo")
        nc.vector.tensor_tensor(
            out=yo[:, :nb, :], in0=y[:, :nb, :],
            in1=cst["lnb"].unsqueeze(1).broadcast_to([P, nb, P]),
            op=mybir.AluOpType.add)
        nc.sync.dma_start(
            out=out_d[j0 * P:(j0 + nb) * P, :].rearrange("(a p) h -> p a h", p=P),
            in_=yo[:, :nb, :])

    # ---- main edge loop, stripes interleaved ----
    gch = None
    gch_t0 = 0
    ci = 0
    ein_t = None
    stripe = 0                      # stripe whose psum tile is being filled
    agg_ps = psS.tile([P, SW * P], F32, space="PSUM", tag="agg")
    open_banks(agg_ps, min(SW, NBLK))
    for g in range(NT // GRP):
        t0 = g * GRP
        if ci < len(chunks) and chunks[ci][1] == t0:
            b, ts, nt = chunks[ci]
            ci += 1
            gch = gp.tile([P, CH_TILES, H], BF16, tag="gch")
            gch_t0 = ts
            base = b * BUCKET
            rows = min(BUCKET, N - base)
            nc.gpsimd.dma_gather(
                out_ap=gch[:, :nt, :],
                in_ap=hbf_d[base:base + rows, :],
                idxs_ap=gidx_sb[:, ts * 8:(ts + nt) * 8],
                num_idxs=nt * P,
                num_idxs_reg=nt * P,
                elem_size=H,
            )
        if g % EB == 0:
            ein_t = ep.tile([TRUNC, EB * GRP * P], BF16, tag="ein")
            nc.sync.dma_start(
                out=ein_t[:],
                in_=einT_d[:, t0 * P:(t0 + EB * GRP) * P])
        eo = (g % EB) * GRP * P

        p1 = ps1.tile([P, GRP * P], F32, space="PSUM", tag="p1")
        nc.tensor.matmul(out=p1[:], lhsT=cst["W1e"],
                         rhs=ein_t[:, eo:eo + GRP * P],
                         start=True, stop=True, skip_group_check=True)
        hidT = ep.tile([P, GRP * P], BF16, tag="hidT")
        nc.scalar.activation(out=hidT[:], in_=p1[:],
                             func=mybir.ActivationFunctionType.Silu,
                             bias=cst["b1e"])

        pw = psw.tile([P, GRP * P], F32, space="PSUM", tag="pw")
        opener(pw[:], rhs_ap=cst["b2e4"], lhs_ap=cst["ocol"])
        for i in range(GRP):
            nc.tensor.matmul(out=pw[:, i * P:(i + 1) * P],
                             lhsT=hidT[:, i * P:(i + 1) * P],
                             rhs=cst["W2e"],
                             start=False, stop=(i == GRP - 1),
                             skip_group_check=True)

        msg = ep.tile([P, GRP * P], BF16, tag="msg")
        o = t0 - gch_t0
        gin = gch[:, o:o + GRP, :].rearrange("p a b -> p (a b)")
        nc.vector.tensor_tensor(out=msg[:], in0=pw[:], in1=gin,
                                op=mybir.AluOpType.mult)

        s4 = sp_.tile([P, GRP * P], BF16, tag="S")
        for i in range(GRP):
            nc.vector.tensor_scalar(
                out=s4[:, i * P:(i + 1) * P], in0=cst["iota"],
                scalar1=dstf_sb[:, t0 + i:t0 + i + 1], scalar2=None,
                op0=mybir.AluOpType.is_equal)

        for i in range(GRP):
            t = t0 + i
            r = troute[t]
            if r is None:               # pure padding tile
                continue
            blk, first, last = r
            poff = blk - stripe * SW
            nc.tensor.matmul(out=agg_ps[:, poff * P:(poff + 1) * P],
                             lhsT=msg[:, i * P:(i + 1) * P],
                             rhs=s4[:, i * P:(i + 1) * P],
                             start=False, stop=(t in stop_tiles),
                             skip_group_check=True)

        if stripe < NSTR and t0 + GRP - 1 >= stripe_done[stripe]:
            # stripe's scatter closed: run its node phase, open next stripe
            b0 = stripe * SW
            nb_total = min(SW, NBLK - b0)
            q = 0
            while q < nb_total:
                nb = min(NQ, nb_total - q)
                node_quad(b0 + q, nb, agg_ps, q)
                q += nb
            stripe += 1
            if stripe < NSTR:
                agg_ps = psS.tile([P, SW * P], F32, space="PSUM", tag="agg")
                open_banks(agg_ps, min(SW, NBLK - stripe * SW))


def _prepare(cfg, h, edge_index, edge_attr, edge_sh,
             W1e, b1e, W2e, b2e, W1n, b1n, W2n, b2n, ln_g, ln_b,
             loop=1):
    meta, per_core, hbf = _prep(cfg, h, edge_index, edge_attr, edge_sh)

    cpack = np.zeros((P, 512), np.float32)
    cpack[:, 0:128] = np.tile(np.asarray(ln_g, np.float32).reshape(1, -1), (P, 1))
    cpack[:, 128:256] = np.tile(np.asarray(ln_b, np.float32).reshape(1, -1), (P, 1))
    cpack[:, 256] = np.asarray(b1e, np.float32)
    cpack[:, 257] = np.asarray(b1n, np.float32)
    cpack[:, 258] = EPS

    cpbf = np.zeros((P, 3072), np.float32)
    cpbf[:, 0:128] = np.tile(np.arange(P, dtype=np.float32)[None, :], (P, 1))
    cpbf[:, 512:640] = np.asarray(W2e, np.float32)
    W1n32 = np.asarray(W1n, np.float32)
    cpbf[:, 640:768] = W1n32[:H]
    cpbf[:, 768:896] = W1n32[H:]
    cpbf[:, 896:1024] = np.asarray(W2n, np.float32)
    cpbf[0:TRUNC, 1024:1152] = np.asarray(W1e, np.float32)
    # zrow at 1152:1664 stays zero
    cpbf[0, 1664:2176] = np.tile(np.asarray(b2e, np.float32), GRP)
    cpbf[0, 2432:2944] = np.tile(np.asarray(b2n, np.float32), NQ)
    cpbf[0, 2304:2432] = 1.0
    weights = dict(cpack=cpack, cpbf=_bf(cpbf))

    nc = _build(cfg, meta, weights, loop=loop)
    nc.compile()

    in_maps = []
    for c in range(NCORES):
        m = dict(hbf=hbf, **per_core[c], **weights)
        in_maps.append(m)
    return nc, in_maps


def _run(cfg, h, edge_index, edge_attr, edge_sh,
         W1e, b1e, W2e, b2e, W1n, b1n, W2n, b2n, ln_g, ln_b, trace=False):
    nc, in_maps = _prepare(cfg, h, edge_index, edge_attr, edge_sh,
                           W1e, b1e, W2e, b2e, W1n, b1n, W2n, b2n, ln_g, ln_b)
    res = run_bass_kernel_spmd(nc, in_maps, list(range(NCORES)), trace=trace)
    out = np.concatenate([res.results[c]["out"] for c in range(NCORES)], axis=0)
    return out[:cfg["N"]], res


def kernel(h, edge_index, edge_attr, edge_sh,
           W1e, b1e, W2e, b2e, W1n, b1n, W2n, b2n, ln_g, ln_b):
    cfg = _full_cfg(N=h.shape[0], E=edge_index.shape[1])
    out, _ = _run(cfg, h, edge_index, edge_attr, edge_sh,
                  W1e, b1e, W2e, b2e, W1n, b1n, W2n, b2n, ln_g, ln_b)
    return out
